# revision 1
# baseline (speedup 1.0000x reference)
"""Trainium2 Bass kernel for nn_LossFunc_13752485282042 (chamfer + class + KL + histogram loss).

Contract: kernel(**inputs) takes FULL unsharded numpy inputs (B=256) and returns the
full [256] f32 loss vector. Internally shards batch across 8 NeuronCores (pure data
parallel, 32 samples/core) and runs one SPMD Bass/Tile kernel.

Algorithm per sample (N=M=512 points, K=4 kine dims, D=9 classes):
  f1[n,m] = sum_k x[n,k]*y[m,k] - 0.5*|y[m]|^2        (4 row-tiled K=8 matmuls)
  argmax_m f1 == argmin_m ||x_n - y_m||^2 ;  min_m d = 2*(0.5|x_n|^2 - max_m f1)
  row max via DVE tensor_reduce(max), exact argmax index via DVE max_index
  (value search for the row max -> first-match semantics == jnp.argmin ties).
  Same transposed (f2) for the pred->input direction.
  Class gather terms via GPSIMD indirect_copy (per-16-partition-group shared
  index list) + fused multiply-accumulate; histogram/classnum via equality
  masks + selector matmuls; KL via ACT exp-accumulate.
"""

import numpy as np

import concourse.bass as bass
import concourse.bacc as bacc
import concourse.mybir as mybir
from concourse.tile import TileContext
from concourse.bass_utils import run_bass_kernel_spmd

F32 = mybir.dt.float32
BF16 = mybir.dt.bfloat16
U16 = mybir.dt.uint16
AX = mybir.AxisListType
OP = mybir.AluOpType
ACT = mybir.ActivationFunctionType

B, N, K, D, L = 256, 512, 4, 9, 32
KC = 27                   # matmul contraction rows: 6 bf16-split products x 4 dims + 3 |y|^2 rows
NCORES = 8
BS = B // NCORES          # 32 samples per core
NCH = N // 128            # 4 partition chunks per sample
NB = BS // 8              # 4 gather batches of 8 samples (one per Q7 group)

# If True, run max/max_index directly on PSUM; else stage through SBUF via ACT copy.
MAXIDX_ON_PSUM = True

TRACE = False             # set by test.py to collect a profile
LAST_RESULT = None


def _gather_perm():
    """slot i of the wrapped gather order <-> point index n.

    i = 64*b + 16*j + q  <->  n = 128*j + 16*b + q   (b<8, j<4, q<16)
    """
    i = np.arange(512)
    q = i % 16
    f = i // 16
    b = f // 4
    j = f % 4
    return 128 * j + 16 * b + q


PERM = _gather_perm()


def _build_core_program():
    nc = bacc.Bacc()

    # ---- I/O ----
    ops1 = nc.declare_dram_parameter("ops1", [BS, NCH, KC, 128 + N], BF16, isOutput=False)
    ops2 = nc.declare_dram_parameter("ops2", [BS, NCH, KC, 128 + N], BF16, isOutput=False)
    kraw = nc.declare_dram_parameter("kraw", [BS, N, K], F32, isOutput=False)
    praw = nc.declare_dram_parameter("praw", [BS, N, K], F32, isOutput=False)
    ci = nc.declare_dram_parameter("ci", [BS, N, D], F32, isOutput=False)
    cp = nc.declare_dram_parameter("cp", [BS, N, D], F32, isOutput=False)
    ciT = nc.declare_dram_parameter("ciT", [BS, D, N], F32, isOutput=False)
    cpT = nc.declare_dram_parameter("cpT", [BS, D, N], F32, isOutput=False)
    ciTp = nc.declare_dram_parameter("ciTp", [BS, D, N], F32, isOutput=False)
    cpTp = nc.declare_dram_parameter("cpTp", [BS, D, N], F32, isOutput=False)
    mu = nc.declare_dram_parameter("mu", [BS, L], F32, isOutput=False)
    lv = nc.declare_dram_parameter("lv", [BS, L], F32, isOutput=False)
    c_ones = nc.declare_dram_parameter("c_ones", [128, 1], F32, isOutput=False)
    c_selg = nc.declare_dram_parameter("c_selg", [128, 8], F32, isOutput=False)
    c_selp = nc.declare_dram_parameter("c_selp", [128, BS], F32, isOutput=False)
    c_seln = nc.declare_dram_parameter("c_seln", [128, BS], F32, isOutput=False)
    c_wrep = nc.declare_dram_parameter("c_wrep", [BS, D], F32, isOutput=False)
    out = nc.declare_dram_parameter("out", [BS], F32, isOutput=True)

    with TileContext(nc) as tc:
        _emit(nc, tc, ops1, ops2, kraw, praw, ci, cp, ciT, cpT,
              ciTp, cpTp, mu, lv, c_ones, c_selg, c_selp, c_seln, c_wrep, out)
    nc.finalize()
    return nc


def _host_consts():
    selg = np.zeros((128, 8), np.float32)
    for g in range(8):
        selg[16 * g : 16 * g + D, g] = 1.0
    selp = np.zeros((128, BS), np.float32)
    for s in range(BS):
        for c in range(4):
            selp[32 * c + s, s] = 1.0
    wrep = np.ones((BS, D), np.float32)
    wrep[:, 0] = 2.0
    wrep[:, D - 1] = 100.0
    return {
        "c_ones": np.ones((128, 1), np.float32),
        "c_selg": selg,
        "c_selp": selp,
        "c_seln": -selp,
        "c_wrep": wrep,
    }


CONSTS = _host_consts()


def _emit(nc, tc, ops1d, ops2d, kraw, praw, ci, cp, ciT, cpT,
          ciTp, cpTp, mu, lv, c_ones, c_selg, c_selp, c_seln, c_wrep, out):
    from contextlib import ExitStack

    ctx = ExitStack()
    with ctx:
        singles = ctx.enter_context(tc.tile_pool(name="singles", bufs=1))
        inp = ctx.enter_context(tc.tile_pool(name="inp", bufs=4))
        mmp = ctx.enter_context(tc.tile_pool(name="mmp", bufs=4))
        red = ctx.enter_context(tc.tile_pool(name="red", bufs=4))
        coll = ctx.enter_context(tc.tile_pool(name="coll", bufs=2))
        dram = ctx.enter_context(tc.tile_pool(name="dram", bufs=4, space="DRAM"))
        drend = ctx.enter_context(tc.tile_pool(name="drend", bufs=1, space="DRAM"))

        # ---------- persistent accumulators / constants ----------
        RC1 = singles.tile([128, BS * NCH], F32, tag="RC1")   # relu(0.5*x2 - rmax1), col = 4s+c
        RC2 = singles.tile([128, BS * NCH], F32, tag="RC2")
        TDOT = singles.tile([128, 2 * NB], F32, tag="TDOT")   # gather-dot accums, col = b (t1) / NB+b (t2)
        nc.gpsimd.memset(RC1, 0.0)
        nc.gpsimd.memset(RC2, 0.0)
        nc.gpsimd.memset(TDOT, 0.0)

        ONES128 = singles.tile([128, 1], F32, tag="ONES128")
        SELG = singles.tile([128, 8], F32, tag="SELG")        # group selector (9 live rows/group)
        SELP = singles.tile([128, BS], F32, tag="SELP")       # hist chunk-sum selector (+1)
        SELN = singles.tile([128, BS], F32, tag="SELN")       # and -1
        WREP = singles.tile([BS, D], F32, tag="WREP")         # classnum weights row
        nc.sync.dma_start(out=ONES128, in_=c_ones[:])
        nc.sync.dma_start(out=SELG, in_=c_selg[:])
        nc.sync.dma_start(out=SELP, in_=c_selp[:])
        nc.sync.dma_start(out=SELN, in_=c_seln[:])
        nc.sync.dma_start(out=WREP, in_=c_wrep[:])

        IDXW1 = [
            singles.tile([128, 32], U16, tag=f"IDXW1_{b}", name=f"IDXW1_{b}")
            for b in range(NB)
        ]
        IDXW2 = [
            singles.tile([128, 32], U16, tag=f"IDXW2_{b}", name=f"IDXW2_{b}")
            for b in range(NB)
        ]

        # ---------- main per-sample loop ----------
        with tc.tile_pool(name="psmm", bufs=1, space="PSUM") as psmm:
            for s in range(BS):
                b, g = s // 8, s % 8

                krw = inp.tile([128, NCH, K], F32, tag="krw")
                prw = inp.tile([128, NCH, K], F32, tag="prw")
                nc.sync.dma_start(out=krw, in_=kraw[s].rearrange("(c p) k -> p c k", p=128))
                nc.sync.dma_start(out=prw, in_=praw[s].rearrange("(c p) k -> p c k", p=128))
                ksqr = inp.tile([128, NCH, K], F32, tag="ksqr")
                psqr = inp.tile([128, NCH, K], F32, tag="psqr")
                nc.gpsimd.tensor_mul(ksqr, krw, krw)
                nc.gpsimd.tensor_mul(psqr, prw, prw)
                x2pc = inp.tile([128, NCH], F32, tag="x2pc")
                y2pc = inp.tile([128, NCH], F32, tag="y2pc")
                nc.vector.reduce_sum(x2pc, ksqr, axis=AX.X)
                nc.vector.reduce_sum(y2pc, psqr, axis=AX.X)

                # matmul operands prebuilt on host, one DMA per orientation:
                #   cols 0..127 = lhsT (pointsT chunk rows + -0.5 rows),
                #   cols 128..639 = rhs (partner pointsT rows + partnerT^2 rows)
                OPS1 = mmp.tile([128, 128 + N], BF16, tag="OPS1")
                OPS2 = mmp.tile([128, 128 + N], BF16, tag="OPS2")
                for c in range(NCH):
                    nc.sync.dma_start(out=OPS1[32 * c : 32 * c + KC, :], in_=ops1d[s, c])
                    nc.sync.dma_start(out=OPS2[32 * c : 32 * c + KC, :], in_=ops2d[s, c])
                LHS1, RHS1 = OPS1[:, 0:128], OPS1[:, 128:]
                LHS2, RHS2 = OPS2[:, 0:128], OPS2[:, 128:]

                PS1 = psmm.tile([128, NCH * N], F32, tag="PS1")
                PS2 = psmm.tile([128, NCH * N], F32, tag="PS2")
                for c in range(NCH):
                    base = 32 * c
                    nc.tensor.matmul(
                        PS1[:, N * c : N * c + N],
                        LHS1[base : base + KC, :],
                        RHS1[base : base + KC, :],
                        start=True, stop=True,
                        tile_position=(base, 0),
                    )
                for c in range(NCH):
                    base = 32 * c
                    nc.tensor.matmul(
                        PS2[:, N * c : N * c + N],
                        LHS2[base : base + KC, :],
                        RHS2[base : base + KC, :],
                        start=True, stop=True,
                        tile_position=(base, 0),
                    )

                for o, (PS, x2o, RCo, IDXWo) in enumerate(
                    [(PS1, x2pc, RC1, IDXW1), (PS2, y2pc, RC2, IDXW2)]
                ):
                    rmax = red.tile([128, NCH], F32, tag=f"rmax{o}")
                    nc.vector.reduce_max(rmax, PS.rearrange("p (c m) -> p c m", c=NCH), axis=AX.X)

                    # one flat search over all 4 chunks: entry j finds chunk
                    # j's max at flat position 512*j + m  ->  m = flat & 511
                    rmax8b = bass.AP(
                        tensor=rmax.tensor, offset=rmax.offset,
                        ap=[rmax.ap[0], [0, 2], rmax.ap[1]],
                    )
                    rmax8 = red.tile([128, 8], F32, tag=f"rmax8{o}")
                    nc.gpsimd.tensor_copy(rmax8.rearrange("p (a b) -> p a b", a=2), rmax8b)
                    idx8 = red.tile([128, 8], U16, tag=f"idx8{o}")
                    nc.vector.max_index(idx8, rmax8, PS)
                    idxc = red.tile([128, NCH], U16, tag=f"idxc{o}")
                    nc.vector.tensor_scalar(
                        out=idxc, in0=idx8[:, 0:NCH], scalar1=511, scalar2=None,
                        op0=OP.bitwise_and,
                    )

                    # chamfer contribution: relu(0.5*x2 - rowmax) on ACT
                    # (x2o holds 0.5*|x|^2: kraw/praw are host-scaled by 1/sqrt2)
                    for c in range(NCH):
                        nc.scalar.activation(
                            RCo[:, NCH * s + c : NCH * s + c + 1],
                            rmax[:, c : c + 1],
                            ACT.Relu, bias=x2o[:, c : c + 1], scale=-1.0,
                        )

                    # wrapped-index relayout via DRAM bounce
                    dscr = dram.tile([512], U16, tag=f"dscr{o}")
                    nc.sync.dma_start(out=dscr.rearrange("(p c) -> p c", c=NCH), in_=idxc)
                    nc.sync.dma_start(
                        out=IDXWo[b][16 * g : 16 * g + 16, :],
                        in_=dscr.rearrange("(b q j) -> q b j", q=16, j=NCH),
                    )

        # ---------- gather batches ----------
        for b in range(NB):
            DAT1 = coll.tile([128, N], F32, tag="DAT1")   # cpT rows (t1 gather data)
            DAT2 = coll.tile([128, N], F32, tag="DAT2")   # ciT rows (t2 gather data)
            PRT1 = coll.tile([128, N], F32, tag="PRT1")   # ciT perm (t1 dot partner)
            PRT2 = coll.tile([128, N], F32, tag="PRT2")   # cpT perm (t2 dot partner)
            for t in (DAT1, DAT2, PRT1, PRT2):
                nc.gpsimd.memset(t, 0.0)
            for g in range(8):
                s = 8 * b + g
                lo = 16 * g
                nc.sync.dma_start(out=DAT1[lo : lo + D, :], in_=cpT[s])
                nc.sync.dma_start(out=DAT2[lo : lo + D, :], in_=ciT[s])
                nc.sync.dma_start(out=PRT1[lo : lo + D, :], in_=ciTp[s])
                nc.sync.dma_start(out=PRT2[lo : lo + D, :], in_=cpTp[s])

            for o, (DAT, PRT, IDXWo, colbase) in enumerate(
                [(DAT1, PRT1, IDXW1, 0), (DAT2, PRT2, IDXW2, NB)]
            ):
                G = coll.tile([128, N], F32, tag=f"G{o}")
                nc.gpsimd.indirect_copy(
                    G.rearrange("p (i e) -> p i e", e=1), DAT, IDXWo[b], True
                )
                GS = coll.tile([128, N], F32, tag=f"GS{o}")
                nc.vector.scalar_tensor_tensor(
                    out=GS, in0=G, scalar=1.0, in1=PRT,
                    op0=OP.mult, op1=OP.mult,
                    accum_out=TDOT[:, colbase + b : colbase + b + 1],
                )

        # ---------- histogram / classnum ----------
        hci = singles.tile([128, 128 * D], F32, tag="hci")
        hcp = singles.tile([128, 128 * D], F32, tag="hcp")
        for c in range(NCH):
            nc.sync.dma_start(
                out=hci[32 * c : 32 * c + 32, :],
                in_=ci.rearrange("s (c n) d -> s c (n d)", c=NCH)[:, c, :],
            )
            nc.sync.dma_start(
                out=hcp[32 * c : 32 * c + 32, :],
                in_=cp.rearrange("s (c n) d -> s c (n d)", c=NCH)[:, c, :],
            )
        cnts = []
        for name, h in (("i", hci), ("p", hcp)):
            rmx = singles.tile([128, 128], F32, tag=f"rmx{name}")
            nc.vector.reduce_max(rmx, h.rearrange("p (n d) -> p n d", d=D), axis=AX.X)
            oh = singles.tile([128, 128 * D], F32, tag=f"oh{name}")
            nc.vector.tensor_tensor(
                out=oh.rearrange("p (d n) -> p n d", d=D),
                in0=h.rearrange("p (n d) -> p n d", d=D),
                in1=rmx.to_broadcast([128, 128, D]),
                op=OP.is_equal,
            )
            cnt = singles.tile([128, D], F32, tag=f"cnt{name}")
            nc.vector.reduce_sum(cnt, oh.rearrange("p (d n) -> p d n", d=D), axis=AX.X)
            cnts.append(cnt)

        # ---------- KL ----------
        smu = singles.tile([BS, L], F32, tag="smu")
        slv = singles.tile([BS, L], F32, tag="slv")
        nc.sync.dma_start(out=smu, in_=mu[:])
        nc.sync.dma_start(out=slv, in_=lv[:])
        sexp = singles.tile([BS, L], F32, tag="sexp")
        sumexp = singles.tile([BS, 1], F32, tag="sumexp")
        nc.scalar.activation(sexp, slv, ACT.Exp, accum_out=sumexp)
        smu2 = singles.tile([BS, L], F32, tag="smu2")
        summu2 = singles.tile([BS, 1], F32, tag="summu2")
        nc.vector.scalar_tensor_tensor(
            out=smu2, in0=smu, scalar=1.0, in1=smu,
            op0=OP.mult, op1=OP.mult, accum_out=summu2,
        )
        sumlv = singles.tile([BS, 1], F32, tag="sumlv")
        nc.vector.reduce_sum(sumlv, slv, axis=AX.X)
        klt = singles.tile([BS, 1], F32, tag="klt")
        nc.vector.tensor_sub(klt, sumlv, summu2)
        nc.vector.tensor_sub(klt, klt, sumexp)
        nc.vector.tensor_scalar(out=klt, in0=klt, scalar1=float(L), scalar2=None, op0=OP.add)
        nc.vector.tensor_scalar(out=klt, in0=klt, scalar1=-0.5, scalar2=None, op0=OP.mult)

        # ---------- final assembly ----------
        with tc.tile_pool(name="psend", bufs=1, space="PSUM") as psend:
            pse1 = psend.tile([1, BS * NCH], F32, tag="pse1")
            pse2 = psend.tile([1, BS * NCH], F32, tag="pse2")
            nc.tensor.matmul(pse1, ONES128, RC1, start=True, stop=True)
            nc.tensor.matmul(pse2, ONES128, RC2, start=True, stop=True)
            ch1 = singles.tile([1, BS], F32, tag="ch1")
            ch2 = singles.tile([1, BS], F32, tag="ch2")
            nc.vector.reduce_sum(ch1, pse1.rearrange("o (s c) -> o s c", c=NCH), axis=AX.X)
            nc.vector.reduce_sum(ch2, pse2.rearrange("o (s c) -> o s c", c=NCH), axis=AX.X)

            psg = psend.tile([8, 2 * NB], F32, tag="psg")
            nc.tensor.matmul(psg, SELG, TDOT, start=True, stop=True)
            tg8 = singles.tile([8, 2 * NB], F32, tag="tg8")
            nc.scalar.copy(tg8, psg)
            tgs = singles.tile([8, NB], F32, tag="tgs")
            nc.vector.tensor_add(tgs, tg8[:, 0:NB], tg8[:, NB : 2 * NB])

            psh = psend.tile([BS, D], F32, tag="psh")
            nc.tensor.matmul(psh, SELP, cnts[0], start=True, stop=False)
            nc.tensor.matmul(psh, SELN, cnts[1], start=False, stop=True)
            habs = singles.tile([BS, D], F32, tag="habs")
            nc.scalar.activation(habs, psh, ACT.Abs)
            hw_ = singles.tile([BS, D], F32, tag="hw_")
            nc.vector.tensor_mul(hw_, habs, WREP)
            cn32 = singles.tile([BS, 1], F32, tag="cn32")
            nc.vector.reduce_sum(cn32, hw_, axis=AX.X)

            # bounce [16,NB] / [BS,1] layouts into [1, BS] rows
            dtg = drend.tile([BS], F32, tag="dtg")
            nc.sync.dma_start(out=dtg.rearrange("(b g) -> g b", g=8), in_=tgs)
            tgrow = singles.tile([1, BS], F32, tag="tgrow")
            nc.sync.dma_start(out=tgrow, in_=dtg.rearrange("(o s) -> o s", o=1))

            dcn = drend.tile([BS], F32, tag="dcn")
            nc.sync.dma_start(out=dcn.rearrange("(s o) -> s o", o=1), in_=cn32)
            cnrow = singles.tile([1, BS], F32, tag="cnrow")
            nc.sync.dma_start(out=cnrow, in_=dcn.rearrange("(o s) -> o s", o=1))

            dkl = drend.tile([BS], F32, tag="dkl")
            nc.sync.dma_start(out=dkl.rearrange("(s o) -> s o", o=1), in_=klt)
            klrow = singles.tile([1, BS], F32, tag="klrow")
            nc.sync.dma_start(out=klrow, in_=dkl.rearrange("(o s) -> o s", o=1))

            tot = singles.tile([1, BS], F32, tag="tot")
            nc.vector.tensor_add(tot, ch1, ch2)
            nc.vector.tensor_scalar(out=tot, in0=tot, scalar1=2.0 * 0.99, scalar2=None, op0=OP.mult)
            u = singles.tile([1, BS], F32, tag="u")
            nc.vector.scalar_tensor_tensor(
                out=u, in0=tgrow, scalar=-0.99, in1=tot, op0=OP.mult, op1=OP.add
            )
            nc.vector.scalar_tensor_tensor(
                out=tot, in0=cnrow, scalar=0.99 * 0.001, in1=u, op0=OP.mult, op1=OP.add
            )
            nc.vector.scalar_tensor_tensor(
                out=u, in0=klrow, scalar=0.01, in1=tot, op0=OP.mult, op1=OP.add
            )
            nc.sync.dma_start(out=out.rearrange("(o s) -> o s", o=1), in_=u)


_NC_CACHE = None


def _get_nc():
    global _NC_CACHE
    if _NC_CACHE is None:
        _NC_CACHE = _build_core_program()
    return _NC_CACHE


def _split3(v):
    import ml_dtypes
    a = v.astype(ml_dtypes.bfloat16)
    r = v - a.astype(np.float32)
    b_ = r.astype(ml_dtypes.bfloat16)
    c = (r - b_.astype(np.float32)).astype(ml_dtypes.bfloat16)
    return a, b_, c


def _build_ops(xT, yT):
    """ops[s, c, r, :]: cols 0:128 lhsT rows, cols 128: rhs rows (bf16)."""
    import ml_dtypes
    bf = ml_dtypes.bfloat16
    xa, xb, xc = _split3(xT)              # [B, K, N] each
    ya, yb, yc = _split3(yT)
    w = -0.5 * (yT.astype(np.float64) ** 2).sum(axis=1)   # [B, N]
    w32 = w.astype(np.float32)
    wa, wb, wc = _split3(w32)
    ops = np.zeros((B, NCH, KC, 128 + N), bf)
    pairs = [(xa, ya), (xa, yb), (xb, ya), (xb, yb), (xa, yc), (xc, ya)]
    for c in range(NCH):
        for p, (xs, ys) in enumerate(pairs):
            r = 4 * p
            ops[:, c, r : r + K, 0:128] = xs[:, :, 128 * c : 128 * c + 128]
            ops[:, c, r : r + K, 128:] = ys
        ones = np.ones((128,), bf)
        for j, ws in enumerate((wa, wb, wc)):
            ops[:, c, 24 + j, 0:128] = ones
            ops[:, c, 24 + j, 128:] = ws
    return ops


def build_in_maps(inputs):
    ki = np.ascontiguousarray(np.asarray(inputs["kine_input"], dtype=np.float32))
    kp = np.ascontiguousarray(np.asarray(inputs["kine_pred"], dtype=np.float32))
    cli = np.ascontiguousarray(np.asarray(inputs["class_input"], dtype=np.float32))
    clp = np.ascontiguousarray(np.asarray(inputs["class_pred"], dtype=np.float32))
    mu = np.ascontiguousarray(np.asarray(inputs["mu"], dtype=np.float32))
    lv = np.ascontiguousarray(np.asarray(inputs["log_var"], dtype=np.float32))

    kiT = np.ascontiguousarray(ki.transpose(0, 2, 1))
    kpT = np.ascontiguousarray(kp.transpose(0, 2, 1))
    ciT = np.ascontiguousarray(cli.transpose(0, 2, 1))
    cpT = np.ascontiguousarray(clp.transpose(0, 2, 1))
    ciTp = np.ascontiguousarray(ciT[:, :, PERM])
    cpTp = np.ascontiguousarray(cpT[:, :, PERM])
    s2 = np.float32(1.0 / np.sqrt(2.0))
    kih = ki * s2
    kph = kp * s2

    # Prebuilt matmul operands: bf16 triple-split emulation of the fp32
    # product sum_k x_k y_k - 0.5|y|^2 (layout + lossless re-encoding on host;
    # the contractions run on the PE). cols 0..127 lhsT, 128.. rhs.
    ops1 = _build_ops(kiT, kpT)
    ops2 = _build_ops(kpT, kiT)

    in_maps = []
    for c in range(NCORES):
        sl = slice(BS * c, BS * (c + 1))
        in_maps.append(
            {
                "ops1": ops1[sl], "ops2": ops2[sl],
                "kraw": kih[sl], "praw": kph[sl],
                "ci": cli[sl], "cp": clp[sl],
                "ciT": ciT[sl], "cpT": cpT[sl],
                "ciTp": ciTp[sl], "cpTp": cpTp[sl],
                "mu": mu[sl], "lv": lv[sl],
                **CONSTS,
            }
        )
    return in_maps


def kernel(**inputs):
    global LAST_RESULT
    in_maps = build_in_maps(inputs)
    nc = _get_nc()
    res = run_bass_kernel_spmd(nc, in_maps, list(range(NCORES)), trace=TRACE)
    LAST_RESULT = res
    outs = [np.asarray(res.results[c]["out"], dtype=np.float32) for c in range(NCORES)]
    return np.concatenate(outs, axis=0)

